# revision 2
# baseline (speedup 1.0000x reference)
"""DeepSeek-MoE layer as a Bass/Tile kernel on 8 Trainium2 NeuronCores.

Strategy (expert-parallel, dense):
  - Each core holds 4 of the 32 routed experts (weights sharded on host) plus
    a 1/8 tensor-parallel slice (352 rows, padded to 384) of the shared expert's
    intermediate dimension.
  - The router (gate matmul fp32 + sigmoid + grouped top-6) is replicated on
    every core; it is exact w.r.t. the fp32 reference selection.
  - Expert GEMMs run in bf16 (fp32 PSUM accumulation). The per-token combine
    weights are folded into the activations before the down-projection, so the
    down-proj GEMMs of all local experts + shared slice accumulate directly in
    PSUM per 128-row output chunk.
  - Partial outputs [H, T] are summed across cores with an on-device
    ReduceScatter; each core returns its 256-row H-slice, the host concatenates
    and transposes.

kernel(**inputs) takes the full unsharded inputs and returns the full output.
"""

import numpy as np
import ml_dtypes

# ---- model dims (hardcoded per problem spec) ----
T = 512          # tokens
H = 2048         # hidden
E = 32           # routed experts
G = 8            # groups
GS = E // G      # experts per group = 4
TKG = 4          # top-k groups
TOPK = 6         # experts per token
I = 1408         # moe intermediate
SCALE = 2.5
NCORES = 8
EL = E // NCORES       # local experts = 4
KC = H // 128          # 16 h-chunks
IM = I // 128          # 11 i-chunks per expert
MH = KC                # 16 output h-chunks
NT = T // 128          # 4 token tiles
M13 = 2 * IM           # 22 w13 m-chunks
SH = (2 * I) // NCORES     # shared slice = 352
SHP = 384                  # padded shared slice
SHC = SHP // 128           # 3 chunks
BIG = 1.0e5

_CACHE = {}


def _build_nc():
    import concourse.bacc as bacc
    import concourse.mybir as mybir
    import concourse.tile as tile
    from concourse.masks import make_identity

    F32 = mybir.dt.float32
    BF16 = mybir.dt.bfloat16
    ALU = mybir.AluOpType
    AFT = mybir.ActivationFunctionType
    AX = mybir.AxisListType

    nc = bacc.Bacc("TRN2", target_bir_lowering=False, debug=False,
                   enable_asserts=True, num_devices=NCORES)

    xT_d = nc.dram_tensor("xT", [H, T], F32, kind="ExternalInput").ap()
    gw_d = nc.dram_tensor("gw", [H, E], F32, kind="ExternalInput").ap()
    cb_d = nc.dram_tensor("cb", [1, E], F32, kind="ExternalInput").ap()
    sel_d = nc.dram_tensor("sel", [E, EL * 128], F32, kind="ExternalInput").ap()
    w13_d = nc.dram_tensor("w13t", [EL, M13, 128, KC, 128], BF16, kind="ExternalInput").ap()
    w2_d = nc.dram_tensor("w2t", [EL, MH, 128, IM, 128], BF16, kind="ExternalInput").ap()
    sg_d = nc.dram_tensor("sgt", [SHC, 128, KC, 128], BF16, kind="ExternalInput").ap()
    su_d = nc.dram_tensor("sut", [SHC, 128, KC, 128], BF16, kind="ExternalInput").ap()
    swd_d = nc.dram_tensor("swdt", [MH, 128, SHC, 128], BF16, kind="ExternalInput").ap()
    out_d = nc.dram_tensor("out", [H // NCORES, T], F32, kind="ExternalOutput").ap()

    with tile.TileContext(nc) as tc:
        with tc.tile_pool(name="per", bufs=1) as per, \
             tc.tile_pool(name="rt", bufs=1) as rt, \
             tc.tile_pool(name="acts", bufs=1) as acts, \
             tc.tile_pool(name="wstream", bufs=4) as wstream, \
             tc.tile_pool(name="w2stream", bufs=6) as w2stream, \
             tc.tile_pool(name="ep", bufs=3) as ep, \
             tc.tile_pool(name="ps", bufs=4, space="PSUM") as ps, \
             tc.tile_pool(name="ps2", bufs=2, space="PSUM") as ps2, \
             tc.tile_pool(name="dram", bufs=1, space="DRAM") as dram:

            # ---------- loads ----------
            xT = per.tile([128, KC, T], F32)
            xT_bf = per.tile([128, KC, T], BF16)
            for k in range(KC):
                nc.sync.dma_start(xT[:, k, :], xT_d[k * 128:(k + 1) * 128, :])
                nc.vector.tensor_copy(xT_bf[:, k, :], xT[:, k, :])
            gw = per.tile([128, KC, E], F32)
            nc.sync.dma_start(gw[:], gw_d.rearrange("(k p) e -> p k e", p=128))
            cb_row = per.tile([1, E], F32)
            nc.sync.dma_start(cb_row[:], cb_d[:])
            sel_sb = per.tile([E, EL * 128], F32)
            nc.sync.dma_start(sel_sb[:], sel_d[:])
            ones_row = per.tile([1, 128], F32)
            nc.vector.memset(ones_row[:], 1.0)
            ident = per.tile([128, 128], F32)
            make_identity(nc, ident)

            # corr_bias broadcast to [128, E] via K=1 matmul
            ps_cb = ps2.tile([128, E], F32, tag="small")
            nc.tensor.matmul(ps_cb[:], ones_row[:], cb_row[:], start=True, stop=True)
            cb_bc = rt.tile([128, E], F32)
            nc.vector.tensor_copy(cb_bc[:], ps_cb[:])

            # ---------- gate GEMM (fp32) ----------
            scores = rt.tile([128, NT, E], F32)
            for i in range(NT):
                pl = ps2.tile([128, E], F32, tag="small")
                for k in range(KC):
                    nc.tensor.matmul(pl[:], xT[:, k, i * 128:(i + 1) * 128],
                                     gw[:, k, :], start=(k == 0), stop=(k == KC - 1))
                nc.scalar.activation(scores[:, i, :], pl[:], AFT.Sigmoid)

            # ---------- grouped top-k routing (exact fp32) ----------
            sfc = rt.tile([128, NT, E], F32)
            nc.vector.tensor_tensor(sfc[:], scores[:],
                                    cb_bc[:, None, :].to_broadcast([128, NT, E]), ALU.add)
            sfc_g = sfc[:].rearrange("p n (g s) -> p n g s", s=GS)
            v = [sfc_g[:, :, :, j] for j in range(GS)]
            grp = rt.tile([128, NT, G], F32)
            gtmp = rt.tile([128, NT, G], F32)
            first = True
            for (a, b) in [(0, 1), (2, 3), (0, 2), (0, 3), (1, 2), (1, 3)]:
                nc.vector.tensor_add(gtmp[:], v[a], v[b])
                if first:
                    nc.vector.tensor_copy(grp[:], gtmp[:])
                    first = False
                else:
                    nc.vector.tensor_max(grp[:], grp[:], gtmp[:])

            gmask = rt.tile([128, NT, G], F32)
            nc.vector.memset(gmask[:], 0.0)
            gm = rt.tile([128, NT], F32)
            gism = rt.tile([128, NT, G], F32)
            for _ in range(TKG):
                nc.vector.tensor_reduce(gm[:], grp[:], AX.X, ALU.max)
                nc.vector.tensor_tensor(gism[:], grp[:],
                                        gm[:, :, None].to_broadcast([128, NT, G]), ALU.is_equal)
                nc.vector.tensor_add(gmask[:], gmask[:], gism[:])
                nc.vector.scalar_tensor_tensor(grp[:], gism[:], -BIG, grp[:], ALU.mult, ALU.add)

            ngmask = rt.tile([128, NT, G], F32)
            nc.vector.tensor_scalar(ngmask[:], gmask[:], -1.0, 1.0, ALU.mult, ALU.add)
            msfc = rt.tile([128, NT, E], F32)
            msfc_g = msfc[:].rearrange("p n (g s) -> p n g s", s=GS)
            nc.vector.scalar_tensor_tensor(
                msfc_g, ngmask[:, :, :, None].to_broadcast([128, NT, G, GS]), -BIG,
                sfc_g, ALU.mult, ALU.add)

            sel = rt.tile([128, NT, E], F32)
            nc.vector.memset(sel[:], 0.0)
            km = rt.tile([128, NT], F32)
            kism = rt.tile([128, NT, E], F32)
            for _ in range(TOPK):
                nc.vector.tensor_reduce(km[:], msfc[:], AX.X, ALU.max)
                nc.vector.tensor_tensor(kism[:], msfc[:],
                                        km[:, :, None].to_broadcast([128, NT, E]), ALU.is_equal)
                nc.vector.tensor_add(sel[:], sel[:], kism[:])
                nc.vector.scalar_tensor_tensor(msfc[:], kism[:], -BIG, msfc[:], ALU.mult, ALU.add)

            wsel = rt.tile([128, NT, E], F32)
            nc.vector.tensor_mul(wsel[:], scores[:], sel[:])
            den = rt.tile([128, NT], F32)
            nc.vector.tensor_reduce(den[:], wsel[:], AX.X, ALU.add)
            rin = rt.tile([128, NT], F32)
            nc.vector.reciprocal(rin[:], den[:])
            nc.vector.tensor_scalar_mul(rin[:], rin[:], float(SCALE))
            cw = rt.tile([128, NT, E], F32)
            nc.vector.tensor_tensor(cw[:], wsel[:],
                                    rin[:, :, None].to_broadcast([128, NT, E]), ALU.mult)

            # transpose cw -> cwT [E, T], then per-local-expert broadcast rows
            ps_cwT = ps2.tile([E, T], F32, tag="scratch")
            for i in range(NT):
                nc.tensor.transpose(ps_cwT[:, i * 128:(i + 1) * 128], cw[:, i, :], ident[:])
            cwT = rt.tile([E, T], F32)
            nc.vector.tensor_copy(cwT[:], ps_cwT[:])

            cw_bc = per.tile([128, EL, T], F32)
            for le in range(EL):
                ps_b = ps2.tile([128, T], F32, tag="scratch")
                nc.tensor.matmul(ps_b[:], sel_sb[:, le * 128:(le + 1) * 128], cwT[:],
                                 start=True, stop=True)
                nc.vector.tensor_copy(cw_bc[:, le, :], ps_b[:])

            # ---------- expert GEMM1 + silu + combine-weight fold ----------
            act = acts.tile([128, EL, IM, T], BF16)
            for le in range(EL):
                for im in range(IM):
                    wg = wstream.tile([128, KC, 128], BF16, tag="w13")
                    nc.sync.dma_start(wg[:], w13_d[le, im])
                    wu = wstream.tile([128, KC, 128], BF16, tag="w13")
                    nc.sync.dma_start(wu[:], w13_d[le, IM + im])
                    pg = ps.tile([128, T], F32, tag="mm")
                    pu = ps.tile([128, T], F32, tag="mm")
                    for k in range(KC):
                        nc.tensor.matmul(pg[:], wg[:, k, :], xT_bf[:, k, :],
                                         start=(k == 0), stop=(k == KC - 1))
                    for k in range(KC):
                        nc.tensor.matmul(pu[:], wu[:, k, :], xT_bf[:, k, :],
                                         start=(k == 0), stop=(k == KC - 1))
                    sil = ep.tile([128, T], F32, tag="sil")
                    nc.scalar.activation(sil[:], pg[:], AFT.Sigmoid)
                    tm = ep.tile([128, T], F32, tag="tm")
                    nc.vector.tensor_mul(tm[:], sil[:], pg[:])
                    nc.vector.tensor_mul(tm[:], tm[:], pu[:])
                    nc.vector.tensor_mul(act[:, le, im, :], tm[:], cw_bc[:, le, :])

            # shared expert slice GEMM1
            act_sh = acts.tile([128, SHC, T], BF16)
            for im in range(SHC):
                wg = wstream.tile([128, KC, 128], BF16, tag="w13")
                nc.sync.dma_start(wg[:], sg_d[im])
                wu = wstream.tile([128, KC, 128], BF16, tag="w13")
                nc.sync.dma_start(wu[:], su_d[im])
                pg = ps.tile([128, T], F32, tag="mm")
                pu = ps.tile([128, T], F32, tag="mm")
                for k in range(KC):
                    nc.tensor.matmul(pg[:], wg[:, k, :], xT_bf[:, k, :],
                                     start=(k == 0), stop=(k == KC - 1))
                for k in range(KC):
                    nc.tensor.matmul(pu[:], wu[:, k, :], xT_bf[:, k, :],
                                     start=(k == 0), stop=(k == KC - 1))
                sil = ep.tile([128, T], F32, tag="sil")
                nc.scalar.activation(sil[:], pg[:], AFT.Sigmoid)
                tm = ep.tile([128, T], F32, tag="tm")
                nc.vector.tensor_mul(tm[:], sil[:], pg[:])
                nc.vector.tensor_mul(act_sh[:, im, :], tm[:], pu[:])

            # ---------- GEMM2: accumulate all local experts + shared ----------
            rs_in = dram.tile([H, T], F32)
            for mh in range(MH):
                po = ps.tile([128, T], F32, tag="mm")
                for le in range(EL):
                    w2b = w2stream.tile([128, IM, 128], BF16, tag="w2")
                    nc.sync.dma_start(w2b[:], w2_d[le, mh])
                    for ki in range(IM):
                        nc.tensor.matmul(po[:], w2b[:, ki, :], act[:, le, ki, :],
                                         start=(le == 0 and ki == 0), stop=False)
                swdb = w2stream.tile([128, SHC, 128], BF16, tag="swd")
                nc.sync.dma_start(swdb[:], swd_d[mh])
                for ki in range(SHC):
                    nc.tensor.matmul(po[:], swdb[:, ki, :], act_sh[:, ki, :],
                                     start=False, stop=(ki == SHC - 1))
                om = ep.tile([128, T], F32, tag="om")
                nc.vector.tensor_copy(om[:], po[:])
                nc.sync.dma_start(rs_in[mh * 128:(mh + 1) * 128, :], om[:])

            # ---------- cross-core reduce-scatter ----------
            rs_out = dram.tile([H // NCORES, T], F32)
            nc.gpsimd.collective_compute(
                "ReduceScatter", ALU.add,
                replica_groups=[list(range(NCORES))],
                ins=[rs_in.opt()], outs=[rs_out.opt()])
            nc.sync.dma_start(out_d[:], rs_out[:])

    nc.compile()
    return nc


def _prep_in_maps(inputs):
    bf16 = ml_dtypes.bfloat16
    x = np.ascontiguousarray(np.asarray(inputs["hidden_states"], dtype=np.float32))
    gate_w = np.ascontiguousarray(np.asarray(inputs["gate_w"], dtype=np.float32))
    corr_bias = np.asarray(inputs["corr_bias"], dtype=np.float32).reshape(1, E)
    w13 = np.asarray(inputs["w13"])
    w2 = np.asarray(inputs["w2"])
    sgu = np.asarray(inputs["shared_w_gu"])
    swd = np.asarray(inputs["shared_w_down"])

    xT = np.ascontiguousarray(x.T)                      # [H, T]
    w13_bf = w13.astype(bf16)                           # [E, H, 2I]
    w2_bf = w2.astype(bf16)                             # [E, I, H]
    sgu_bf = sgu.astype(bf16)                           # [H, 2*2816]
    swd_bf = swd.astype(bf16)                           # [2816, H]

    in_maps = []
    for c in range(NCORES):
        e0 = c * EL
        # w13 lhsT tiles: [e, m, p(h%128), k(h//128), f(d%128)]
        w13t = np.ascontiguousarray(
            w13_bf[e0:e0 + EL].reshape(EL, KC, 128, M13, 128).transpose(0, 3, 2, 1, 4))
        # w2 lhsT tiles: [e, mh, p(i%128), ki(i//128), f(h%128)]
        w2t = np.ascontiguousarray(
            w2_bf[e0:e0 + EL].reshape(EL, IM, 128, MH, 128).transpose(0, 3, 2, 1, 4))
        # shared gate/up slices padded to SHP rows of intermediate
        g_sl = sgu_bf[:, c * SH:(c + 1) * SH]           # [H, 352]
        u_sl = sgu_bf[:, 2 * I + c * SH:2 * I + (c + 1) * SH]
        g_pad = np.zeros((H, SHP), dtype=bf16); g_pad[:, :SH] = g_sl
        u_pad = np.zeros((H, SHP), dtype=bf16); u_pad[:, :SH] = u_sl
        sgt = np.ascontiguousarray(
            g_pad.reshape(KC, 128, SHC, 128).transpose(2, 1, 0, 3))
        sut = np.ascontiguousarray(
            u_pad.reshape(KC, 128, SHC, 128).transpose(2, 1, 0, 3))
        d_pad = np.zeros((SHP, H), dtype=bf16); d_pad[:SH] = swd_bf[c * SH:(c + 1) * SH]
        swdt = np.ascontiguousarray(
            d_pad.reshape(SHC, 128, MH, 128).transpose(2, 1, 0, 3))
        # selector: sel[k, le*128 + j] = 1 iff k == e0 + le
        sel = np.zeros((E, EL * 128), dtype=np.float32)
        for le in range(EL):
            sel[e0 + le, le * 128:(le + 1) * 128] = 1.0
        in_maps.append({
            "xT": xT, "gw": gate_w, "cb": corr_bias, "sel": sel,
            "w13t": w13t, "w2t": w2t, "sgt": sgt, "sut": sut, "swdt": swdt,
        })
    return in_maps


def _get_nc():
    if "nc" not in _CACHE:
        _CACHE["nc"] = _build_nc()
    return _CACHE["nc"]


def _run(inputs, trace=False, tmpdir=None):
    from concourse.bass_utils import run_bass_kernel_spmd
    nc = _get_nc()
    in_maps = _prep_in_maps(inputs)
    res = run_bass_kernel_spmd(nc, in_maps, core_ids=list(range(NCORES)),
                               trace=trace, tmpdir=tmpdir)
    outT = np.concatenate([res.results[c]["out"] for c in range(NCORES)], axis=0)
    out = np.ascontiguousarray(outT.T).astype(np.float32)
    return out, res


def kernel(**inputs) -> np.ndarray:
    out, _ = _run(inputs, trace=False)
    return out
